# revision 3
# baseline (speedup 1.0000x reference)
"""MLA decode paged attention (flat_pa_mla latent-cache path) on 8 TRN2 NeuronCores.

Sharding: data-parallel over the block/batch axis. Blocks are grouped 16-per-request
(asserted), so each core gets 4 complete requests = 64 blocks and computes its slice
of the output independently — no collectives.

Host prep per core (the "paged per device" part of the sharding):
  kt [16, 577, 512]: for each qk-group g (4 blocks), rows 0..575 = kv[block].T for the
      4 blocks side by side (d on rows, 4*128 block-positions on cols); row 576 is the
      block_bias row, so the bias addition folds into the QK matmul as one extra
      contraction row against a constant-1.0 row in qt.
  v  [64, 128, 512]: natural-layout value pages (first 512 of the latent dim).
  qt [4, 577, 16]: per-request SCALE*query transposed, with a trailing 1.0 row.

Device (per core): pass A per qk-group: 5 PE matmuls (lhsT = qt chunk [<=128,16],
rhs = kt tile [<=128,512]) accumulate attn [16,512] in PSUM; DVE per-block max,
ACT exp(bias=-max) with fused fp32 row-sum. Per request: grouped max/sum combine
to rescale [16,16]. Pass B per block: scale p, PE-transpose to [128,16], PV matmul
(lhsT=p^T, rhs=v page) accumulating o [16,512] per request in PSUM; DMA out.
"""

import numpy as np

import concourse.bass as bass
import concourse.mybir as mybir
import concourse.tile as tile
from concourse import bacc
from concourse.bass_utils import run_bass_kernel_spmd
from concourse.masks import make_identity

B = 32
H = 16
KVL = 512
ROPE = 64
D = KVL + ROPE          # 576
BS = 128
BPS = 16                # blocks per request
NB = B * BPS            # 512
SCALE = 192 ** -0.5
NCORES = 8
RPC = B // NCORES       # 4 requests per core
NBLK = RPC * BPS        # 64 blocks per core
BPG = 4                 # blocks per qk-group (one N=512 matmul)
NG = NBLK // BPG        # 16 qk-groups per core
DR = D + 1              # 577 rows: 576 latent+rope dims + 1 bias row

# KV page dtype on device. float32 = exact; bfloat16 halves HBM traffic.
KV_DT = mybir.dt.float32
P_DT = mybir.dt.float32  # dtype of stored softmax numerators (PV lhsT)

TRACE = False           # set True (with profhook installed) to NTFF-profile
LAST_RESULTS = None     # BassKernelResults of the last kernel() call when TRACE

_NC_CACHE = {}


def _np_of(dt):
    import ml_dtypes

    return {mybir.dt.float32: np.float32, mybir.dt.bfloat16: ml_dtypes.bfloat16}[dt]


def _build(kv_dt, p_dt):
    f32 = mybir.dt.float32
    nc = bacc.Bacc("TRN2", target_bir_lowering=False, debug=False)
    kt = nc.dram_tensor("kt", [NG, DR, BPG * BS], kv_dt, kind="ExternalInput").ap()
    v = nc.dram_tensor("v", [NBLK, BS, KVL], kv_dt, kind="ExternalInput").ap()
    qt = nc.dram_tensor("qt", [RPC, DR, H], f32, kind="ExternalInput").ap()
    o = nc.dram_tensor("o", [RPC, H, KVL], f32, kind="ExternalOutput").ap()

    with tile.TileContext(nc) as tc:
        with (
            tc.tile_pool(name="singles", bufs=1) as singles,
            tc.tile_pool(name="ktp", bufs=3) as ktp,
            tc.tile_pool(name="vp", bufs=3) as vp,
            tc.tile_pool(name="pp", bufs=4) as pp,
            tc.tile_pool(name="outp", bufs=2) as outp,
            tc.tile_pool(name="stats", bufs=10) as stats,
            tc.tile_pool(name="pap", bufs=3, space="PSUM") as pap,
            tc.tile_pool(name="ptpp", bufs=3, space="PSUM") as ptpp,
            tc.tile_pool(name="pop", bufs=2, space="PSUM") as pop,
        ):
            ident = singles.tile([H, H], p_dt)
            make_identity(nc, ident)

            # q^T chunks: qt1[dlo, r, c, h] covers rows c*128+dlo (c<4);
            # qt2[dlo, r, h] covers rows 512+dlo (rope + bias row).
            qt1 = singles.tile([128, RPC, 4, H], f32)
            qt2 = singles.tile([DR - 512, RPC, H], f32)
            for r in range(RPC):
                nc.sync.dma_start(
                    out=qt1[:, r, :, :],
                    in_=qt[r, 0 : 4 * 128, :].rearrange("(c p) h -> p c h", p=128),
                )
                nc.sync.dma_start(out=qt2[:, r, :], in_=qt[r, 512:DR, :])

            # softmax numerators for the whole core, written in pass A,
            # consumed (rescaled) in pass B
            p_all = singles.tile([H, NBLK, BS], p_dt)

            for r in range(RPC):
                bm = stats.tile([H, BPS], f32)     # per-block max
                nbm = stats.tile([H, BPS], f32)    # negated
                sums = stats.tile([H, BPS], f32)   # per-block sum of exp

                # ---- pass A: QK + per-block softmax stats ----
                for gq in range(NG // RPC):
                    g = r * (NG // RPC) + gq
                    ktile = ktp.tile([128, 5, BPG * BS], kv_dt)
                    nc.sync.dma_start(
                        out=ktile[:, 0:4, :],
                        in_=kt[g, 0 : 4 * 128, :].rearrange(
                            "(c p) s -> p c s", p=128
                        ),
                    )
                    nc.sync.dma_start(
                        out=ktile[0 : DR - 512, 4, :], in_=kt[g, 512:DR, :]
                    )

                    pa = pap.tile([H, BPG * BS], f32)
                    for c in range(4):
                        nc.tensor.matmul(
                            pa,
                            qt1[:, r, c, :],
                            ktile[:, c, :],
                            start=(c == 0),
                            stop=False,
                        )
                    nc.tensor.matmul(
                        pa,
                        qt2[:, r, :],
                        ktile[0 : DR - 512, 4, :],
                        start=False,
                        stop=True,
                    )

                    bsl = slice(BPG * gq, BPG * (gq + 1))
                    nc.vector.reduce_max(
                        out=bm[:, bsl],
                        in_=pa.rearrange("h (j s) -> h j s", j=BPG),
                        axis=mybir.AxisListType.X,
                    )
                    nc.vector.tensor_scalar_mul(nbm[:, bsl], bm[:, bsl], -1.0)
                    for j in range(BPG):
                        idx = BPG * gq + j
                        nc.scalar.activation(
                            out=p_all[:, BPS * r + idx, :],
                            in_=pa[:, BS * j : BS * (j + 1)],
                            func=mybir.ActivationFunctionType.Exp,
                            bias=nbm[:, idx : idx + 1],
                            scale=1.0,
                            accum_out=sums[:, idx : idx + 1],
                        )

                # ---- combine: grouped max/sum -> rescale [H, BPS] ----
                gm = stats.tile([H, 1], f32)
                ngm = stats.tile([H, 1], f32)
                adj = stats.tile([H, BPS], f32)
                sa = stats.tile([H, BPS], f32)
                gs = stats.tile([H, 1], f32)
                gsm = stats.tile([H, BPS], f32)
                rgs = stats.tile([H, BPS], f32)
                resc = stats.tile([H, BPS], f32)
                nc.vector.reduce_max(out=gm, in_=bm, axis=mybir.AxisListType.X)
                nc.vector.tensor_scalar_mul(ngm, gm, -1.0)
                nc.scalar.activation(
                    out=adj,
                    in_=bm,
                    func=mybir.ActivationFunctionType.Exp,
                    bias=ngm[:, 0:1],
                    scale=1.0,
                )
                nc.vector.tensor_mul(sa, sums, adj)
                nc.vector.reduce_sum(out=gs, in_=sa, axis=mybir.AxisListType.X)
                nc.vector.tensor_scalar_max(gsm, sa, gs[:, 0:1])
                nc.vector.reciprocal(rgs, gsm)
                nc.vector.tensor_mul(resc, adj, rgs)

                # ---- pass B: rescale p, transpose, PV accumulate ----
                po = pop.tile([H, KVL], f32)
                for gq in range(NG // RPC):
                    vtile = vp.tile([128, BPG, KVL], kv_dt)
                    n0 = NBLK // RPC * r + BPG * gq
                    nc.sync.dma_start(
                        out=vtile,
                        in_=v[n0 : n0 + BPG, :, :].rearrange("n s e -> s n e"),
                    )
                    for j in range(BPG):
                        idx = BPG * gq + j
                        ps = pp.tile([H, BS], p_dt)
                        nc.vector.tensor_scalar_mul(
                            ps, p_all[:, BPS * r + idx, :], resc[:, idx : idx + 1]
                        )
                        ptp = ptpp.tile([BS, H], p_dt)
                        nc.tensor.transpose(ptp, ps, ident)
                        pt_sb = pp.tile([BS, H], kv_dt)
                        nc.vector.tensor_copy(pt_sb, ptp)
                        nc.tensor.matmul(
                            po,
                            pt_sb,
                            vtile[:, j, :],
                            start=(idx == 0),
                            stop=(idx == BPS - 1),
                        )
                o_sb = outp.tile([H, KVL], f32)
                nc.scalar.copy(o_sb, po)
                nc.sync.dma_start(out=o[r], in_=o_sb)

    nc.compile()
    return nc


def _get_nc():
    key = (KV_DT, P_DT)
    if key not in _NC_CACHE:
        _NC_CACHE[key] = _build(*key)
    return _NC_CACHE[key]


def kernel(query, key_cache, block_mapping, block_bias, block_list, block_groups):
    global LAST_RESULTS
    query = np.asarray(query)
    key_cache = np.asarray(key_cache)
    block_bias = np.asarray(block_bias)
    block_list = np.asarray(block_list)
    block_groups = np.asarray(block_groups)

    # Sort blocks by request; each request must own exactly BPS blocks.
    perm = np.argsort(block_groups, kind="stable")
    bg = block_groups[perm]
    assert (np.bincount(bg, minlength=B) == BPS).all()
    bl = block_list[perm]
    bias = block_bias[perm].astype(np.float32)

    np_kv = _np_of(KV_DT)
    pages = key_cache[bl]  # [NB, BS, D] gathered pages ("paged per device")

    nc = _get_nc()
    in_maps = []
    for c in range(NCORES):
        sl = slice(NBLK * c, NBLK * (c + 1))
        pg = pages[sl]  # [64, 128, 576]
        ktT = pg.transpose(0, 2, 1)  # [64, 576, 128]
        kt = np.empty((NG, DR, BPG * BS), np_kv)
        kt[:, :D, :] = (
            ktT.reshape(NG, BPG, D, BS).transpose(0, 2, 1, 3).reshape(NG, D, BPG * BS)
        )
        kt[:, D, :] = bias[sl].reshape(NG, BPG * BS)
        vv = np.ascontiguousarray(pg[:, :, :KVL]).astype(np_kv)
        qt = np.empty((RPC, DR, H), np.float32)
        qt[:, :D, :] = (SCALE * query[RPC * c : RPC * (c + 1)]).transpose(0, 2, 1)
        qt[:, D, :] = 1.0
        in_maps.append({"kt": kt, "v": vv, "qt": qt})

    res = run_bass_kernel_spmd(nc, in_maps, list(range(NCORES)), trace=TRACE)
    if TRACE:
        LAST_RESULTS = res
    return np.concatenate(
        [res.results[i]["o"] for i in range(NCORES)], axis=0
    ).astype(np.float32)


# revision 7
# speedup vs baseline: 2.1896x; 2.1896x over previous
"""MLA decode paged attention (flat_pa_mla latent-cache path) on 8 TRN2 NeuronCores.

Sharding: data-parallel over the block/batch axis. Blocks are grouped 16-per-request
(asserted), so each core gets 4 complete requests = 64 blocks and computes its slice
of the output independently — no collectives.

Host prep per core (the "paged per device" part of the sharding):
  kt [16, 577, 512]: for each (request r, group i) of 4 blocks, rows 0..575 =
      kv[block].T for the 4 blocks side by side (d on rows, 4*128 block-positions on
      cols); row 576 is the block_bias row, so the bias addition folds into the QK
      matmul as one extra contraction row against a constant-1.0 row in qt.
  v  [64, 128, 512]: natural-layout value pages (first 512 of the latent dim).
  qt [4, 577, 16]: per-request SCALE*query transposed, with a trailing 1.0 row.

Device (per core), 4 requests in lockstep so element-wise work runs on 64 partitions:
  pass A per group index i: per request 5 PE matmuls (lhsT = qt chunk [<=128,16],
  rhs = kt tile [<=128,512]) accumulate attn in pa[16r:16r+16, :] of one PSUM bank
  [64,512]; one DVE per-block max over [64,4,128], one ACT exp(bias=-max) per block
  with fused fp32 row-sum. One combine: grouped max/sum -> rescale [64,16].
  Pass B per block position idx: scale p [64,128], PE-transpose to [128,64], then 4
  PV matmuls (lhsT = p^T columns 16r..16r+16, rhs = v page) accumulating the output
  [64,512] PSUM bank; one copy + one DMA out.
"""

import numpy as np

import concourse.bass as bass
import concourse.mybir as mybir
import concourse.tile as tile
from concourse import bacc
from concourse.bass_utils import run_bass_kernel_spmd
from concourse.masks import make_identity

B = 32
H = 16
KVL = 512
ROPE = 64
D = KVL + ROPE          # 576
BS = 128
BPS = 16                # blocks per request
NB = B * BPS            # 512
SCALE = 192 ** -0.5
NCORES = 8
RPC = B // NCORES       # 4 requests per core
NBLK = RPC * BPS        # 64 blocks per core
BPG = 4                 # blocks per qk-group (one N=512 matmul)
NGR = BPS // BPG        # 4 qk-groups per request
DR = D + 1              # 577 rows: 576 latent+rope dims + 1 bias row
RST = 32                # per-request partition stride (PE col groups are 32-wide)
HP = RPC * RST          # 128 partitions spanned by packed per-request ops

# KV page dtype on device. float32 = exact; bfloat16 halves HBM traffic and
# runs the PE at full rate (fp32 matmul = 2 half-speed passes = 4x cost).
KV_DT = mybir.dt.bfloat16
P_DT = mybir.dt.bfloat16  # dtype of stored softmax numerators (PV lhsT)

TRACE = False           # set True (with profhook installed) to NTFF-profile
LAST_RESULTS = None     # BassKernelResults of the last kernel() call when TRACE

_NC_CACHE = {}


def _np_of(dt):
    import ml_dtypes

    return {mybir.dt.float32: np.float32, mybir.dt.bfloat16: ml_dtypes.bfloat16}[dt]


def _build(kv_dt, p_dt):
    f32 = mybir.dt.float32
    nc = bacc.Bacc("TRN2", target_bir_lowering=False, debug=False)
    kt = nc.dram_tensor(
        "kt", [RPC, NGR, DR, BPG * BS], kv_dt, kind="ExternalInput"
    ).ap()
    v = nc.dram_tensor("v", [NBLK, BS, KVL], kv_dt, kind="ExternalInput").ap()
    qt = nc.dram_tensor("qt", [RPC, DR, H], kv_dt, kind="ExternalInput").ap()
    o = nc.dram_tensor("o", [RPC, H, KVL], f32, kind="ExternalOutput").ap()

    with tile.TileContext(nc) as tc:
        with (
            tc.tile_pool(name="singles", bufs=1) as singles,
            tc.tile_pool(name="ktp", bufs=3) as ktp,
            tc.tile_pool(name="vp", bufs=4) as vp,
            tc.tile_pool(name="pp", bufs=4) as pp,
            tc.tile_pool(name="stats", bufs=10) as stats,
            tc.tile_pool(name="pap", bufs=3, space="PSUM") as pap,
            tc.tile_pool(name="ptpp", bufs=3, space="PSUM") as ptpp,
            tc.tile_pool(name="pop", bufs=1, space="PSUM") as pop,
        ):
            # kt DMAs first: they are the critical path to the first matmul.
            # ktiles[i][r] holds request r's group-i K^T tiles [128, 5 chunks, 512].
            ktiles = {}
            for i in range(NGR):
                for r in range(RPC):
                    ktile = ktp.tile([128, 5, BPG * BS], kv_dt, tag=f"kt{r}")
                    nc.sync.dma_start(
                        out=ktile[:, 0:4, :],
                        in_=kt[r, i, 0 : 4 * 128, :].rearrange(
                            "(c p) s -> p c s", p=128
                        ),
                    )
                    nc.sync.dma_start(
                        out=ktile[0 : DR - 512, 4, :], in_=kt[r, i, 512:DR, :]
                    )
                    ktiles[(i, r)] = ktile

            # q^T chunks on the scalar-engine DMA queue (parallel to kt loads):
            # qt1[dlo, r, c, h] covers rows c*128+dlo (c<4); qt2 covers 512+dlo.
            qt1 = singles.tile([128, RPC, 4, H], kv_dt)
            qt2 = singles.tile([DR - 512, RPC, H], kv_dt)
            for r in range(RPC):
                nc.scalar.dma_start(
                    out=qt1[:, r, :, :],
                    in_=qt[r, 0 : 4 * 128, :].rearrange("(c p) h -> p c h", p=128),
                )
                nc.scalar.dma_start(out=qt2[:, r, :], in_=qt[r, 512:DR, :])

            ident = singles.tile([HP, HP], p_dt)
            make_identity(nc, ident)

            # softmax numerators for the whole core (pass A -> pass B)
            p_all = singles.tile([HP, BPS, BS], p_dt)
            bm = stats.tile([HP, BPS], f32)
            nbm = stats.tile([HP, BPS], f32)
            sums = stats.tile([HP, BPS], f32)

            # ---- pass A: QK + per-block softmax stats ----
            for i in range(NGR):
                pa = pap.tile([HP, BPG * BS], f32)
                for c in range(4):
                    for r in range(RPC):
                        osl = slice(RST * r, RST * r + H)
                        nc.tensor.matmul(
                            pa[osl, :],
                            qt1[:, r, c, :],
                            ktiles[(i, r)][:, c, :],
                            start=(c == 0),
                            stop=False,
                            tile_position=(0, RST * r),
                        )
                for r in range(RPC):
                    osl = slice(RST * r, RST * r + H)
                    nc.tensor.matmul(
                        pa[osl, :],
                        qt2[:, r, :],
                        ktiles[(i, r)][0 : DR - 512, 4, :],
                        start=False,
                        stop=True,
                        tile_position=(0, RST * r),
                    )

                bsl = slice(BPG * i, BPG * (i + 1))
                nc.vector.reduce_max(
                    out=bm[:, bsl],
                    in_=pa.rearrange("h (j s) -> h j s", j=BPG),
                    axis=mybir.AxisListType.X,
                )
                nc.vector.tensor_scalar_mul(nbm[:, bsl], bm[:, bsl], -1.0)
                for j in range(BPG):
                    idx = BPG * i + j
                    nc.scalar.activation(
                        out=p_all[:, idx, :],
                        in_=pa[:, BS * j : BS * (j + 1)],
                        func=mybir.ActivationFunctionType.Exp,
                        bias=nbm[:, idx : idx + 1],
                        scale=1.0,
                        accum_out=sums[:, idx : idx + 1],
                    )

            # ---- combine: grouped max/sum -> rescale [HP, BPS] ----
            gm = stats.tile([HP, 1], f32)
            ngm = stats.tile([HP, 1], f32)
            adj = stats.tile([HP, BPS], f32)
            sa = stats.tile([HP, BPS], f32)
            gs = stats.tile([HP, 1], f32)
            gsm = stats.tile([HP, BPS], f32)
            rgs = stats.tile([HP, BPS], f32)
            resc = stats.tile([HP, BPS], f32)
            nc.vector.reduce_max(out=gm, in_=bm, axis=mybir.AxisListType.X)
            nc.vector.tensor_scalar_mul(ngm, gm, -1.0)
            nc.scalar.activation(
                out=adj,
                in_=bm,
                func=mybir.ActivationFunctionType.Exp,
                bias=ngm[:, 0:1],
                scale=1.0,
            )
            nc.vector.tensor_mul(sa, sums, adj)
            nc.vector.reduce_sum(out=gs, in_=sa, axis=mybir.AxisListType.X)
            nc.vector.tensor_scalar_max(gsm, sa, gs[:, 0:1])
            nc.vector.reciprocal(rgs, gsm)
            nc.vector.tensor_mul(resc, adj, rgs)

            # ---- pass B: rescale p, transpose, PV accumulate ----
            po = pop.tile([HP, KVL], f32)
            for idx in range(BPS):
                vtile = vp.tile([128, RPC, KVL], kv_dt)
                nc.sync.dma_start(
                    out=vtile,
                    in_=v.rearrange("(r i) s e -> i s r e", r=RPC)[idx],
                )
                ps = pp.tile([HP, BS], p_dt)
                nc.vector.tensor_scalar_mul(
                    ps, p_all[:, idx, :], resc[:, idx : idx + 1]
                )
                ptp = ptpp.tile([BS, HP], p_dt)
                nc.tensor.transpose(ptp, ps, ident)
                pt_sb = pp.tile([BS, HP], kv_dt)
                nc.vector.tensor_copy(pt_sb, ptp)
                for r in range(RPC):
                    nc.tensor.matmul(
                        po[RST * r : RST * r + H, :],
                        pt_sb[:, RST * r : RST * r + H],
                        vtile[:, r, :],
                        start=(idx == 0),
                        stop=(idx == BPS - 1),
                        tile_position=(0, RST * r),
                    )
            o_sb = singles.tile([HP, KVL], f32)
            nc.scalar.copy(o_sb, po)
            for r in range(RPC):
                nc.sync.dma_start(
                    out=o[r], in_=o_sb[RST * r : RST * r + H, :]
                )

    nc.compile()
    return nc


def _get_nc():
    key = (KV_DT, P_DT)
    if key not in _NC_CACHE:
        _NC_CACHE[key] = _build(*key)
    return _NC_CACHE[key]


def kernel(query, key_cache, block_mapping, block_bias, block_list, block_groups):
    global LAST_RESULTS
    query = np.asarray(query)
    key_cache = np.asarray(key_cache)
    block_bias = np.asarray(block_bias)
    block_list = np.asarray(block_list)
    block_groups = np.asarray(block_groups)

    # Sort blocks by request; each request must own exactly BPS blocks.
    perm = np.argsort(block_groups, kind="stable")
    bg = block_groups[perm]
    assert (np.bincount(bg, minlength=B) == BPS).all()
    bl = block_list[perm]
    bias = block_bias[perm].astype(np.float32)

    np_kv = _np_of(KV_DT)
    pages = key_cache[bl]  # [NB, BS, D] gathered pages ("paged per device")

    nc = _get_nc()
    in_maps = []
    for c in range(NCORES):
        sl = slice(NBLK * c, NBLK * (c + 1))
        pg = pages[sl]  # [64, 128, 576]
        ktT = pg.transpose(0, 2, 1)  # [64, 576, 128]
        kt = np.empty((RPC, NGR, DR, BPG * BS), np_kv)
        kt[:, :, :D, :] = (
            ktT.reshape(RPC, NGR, BPG, D, BS)
            .transpose(0, 1, 3, 2, 4)
            .reshape(RPC, NGR, D, BPG * BS)
        )
        kt[:, :, D, :] = bias[sl].reshape(RPC, NGR, BPG * BS)
        vv = np.ascontiguousarray(pg[:, :, :KVL]).astype(np_kv)
        qt = np.empty((RPC, DR, H), np_kv)
        qt[:, :D, :] = (SCALE * query[RPC * c : RPC * (c + 1)]).transpose(0, 2, 1)
        qt[:, D, :] = 1.0
        in_maps.append({"kt": kt, "v": vv, "qt": qt})

    res = run_bass_kernel_spmd(nc, in_maps, list(range(NCORES)), trace=TRACE)
    if TRACE:
        LAST_RESULTS = res
    return np.concatenate(
        [res.results[i]["o"] for i in range(NCORES)], axis=0
    ).astype(np.float32)
